# revision 1
# baseline (speedup 1.0000x reference)
"""Trainium2 Bass kernel: segmented statistical moments (mean/var/skew/kurt).

Strategy (8 NeuronCores, one SPMD program):
  - 4096 sorted segments -> 512 consecutive segments per core; host re-packs
    each core's nodes into a fixed program-uniform layout (so the single
    program works on all cores; per-node data differs, addressing does not).
  - Segments grouped into windows of W=32; every window padded to a common
    tile count t_win (multiple of G). Per-node "rel" = idx - window_base
    (-1 for pad slots) is data, is_equal'd against an iota row on the
    vector engine to build a [128 nodes, 32 segs] one-hot.
  - Per 128-node tile: matmul onehot.T @ [x | x^2 | x^3 | x^4] (float32r,
    N=256, 1 cycle/row) accumulates per-segment power sums in PSUM.
    Powers: x^2 on ACT (Square), x^3 on DVE, x^4 split ACT/DVE.
    FP32R matmuls require dst partition base 0, so each window owns a
    [0:32]-partition view; two windows share each PSUM bank (col 0/256).
  - Power sums copied out raw; host finalizes moments (float64) and
    concatenates cores.
"""

import sys

if "/opt/trn_rl_repo" not in sys.path:
    sys.path.insert(0, "/opt/trn_rl_repo")

import numpy as np

N_CORES = 8
B = 4096
C = 64
SEGS_PER_CORE = B // N_CORES      # 512
W = 32                            # segments per window
WINDOWS_PER_CORE = SEGS_PER_CORE // W   # 16
G = 16                            # 128-node tiles per chunk
GC = G * C                        # 1024
CHUNK = 128 * G                   # 2048 node slots per chunk
ACT_X4_G = 7                      # g-slices of x^4 on ACT (rest on DVE)

_prog_cache = {}
TRACE = False
_CACHE_BUST = False


def _round_fp32r(a):
    bits = np.ascontiguousarray(a).view(np.uint32)
    return ((bits + np.uint32(0x800)) & np.uint32(0xFFFFF000)).view(np.float32)


def _split_excess_waits(nc, mybir, max_waits=1):
    """This walrus build allows only one sync-wait per instruction; move
    extra waits onto standalone sequencer EventSemaphore instructions."""
    n = [0]

    def mk(engine, waits):
        wi = mybir.InstEventSemaphore(name=f"xw_{n[0]}", ins=[], outs=[])
        n[0] += 1
        wi.engine = engine
        wi.sync_info = mybir.SyncInfo(on_wait=list(waits), on_update=[])
        return wi

    for bb in nc.main_func.blocks:
        out = []
        for ins in bb.instructions:
            if ins.opcode == "Matmult":
                # one-hot (and zero) weights are exactly {0,1}
                ins.is_weight_onezero = True
            si = ins.sync_info
            if (
                si is not None
                and len(si.on_wait) > max_waits
                and ins.opcode != "EventSemaphore"
            ):
                waits = list(si.on_wait)
                for w in waits[:-max_waits]:
                    out.append(mk(ins.engine, [w]))
                ins.sync_info = mybir.SyncInfo(
                    on_wait=waits[-max_waits:], on_update=list(si.on_update)
                )
            out.append(ins)
        bb.instructions = out


def _build_program(t_win):
    import concourse.bass as bass
    import concourse.tile as tile
    import concourse.mybir as mybir

    F32 = mybir.dt.float32
    F32R = mybir.dt.float32r

    tiles = WINDOWS_PER_CORE * t_win
    assert tiles % G == 0
    chunks = tiles // G
    rows = tiles * 128
    n_banks = WINDOWS_PER_CORE // 2   # two windows per PSUM bank

    nc = bass.Bass()
    x_d = nc.dram_tensor("x", [rows, C], F32R, kind="ExternalInput")
    rel_d = nc.dram_tensor("rel", [128, tiles], F32, kind="ExternalInput")
    iota_d = nc.dram_tensor("iota", [128, W], F32, kind="ExternalInput")
    out_d = nc.dram_tensor("out", [SEGS_PER_CORE, 4 * C], F32, kind="ExternalOutput")

    with tile.TileContext(nc) as tc:
        with (
            tc.tile_pool(name="const", bufs=1) as const,
            tc.tile_pool(name="powp", bufs=9) as powp,
            tc.tile_pool(name="ohp", bufs=8) as ohp,
            tc.tile_pool(name="relp", bufs=8) as relp,
            tc.tile_pool(name="psp", bufs=1, space="PSUM") as psp,
            tc.tile_pool(name="outp", bufs=3) as outp,
        ):
            iota_t = const.tile([128, W], F32)
            nc.sync.dma_start(out=iota_t[:], in_=iota_d[:])
            zeros_t = const.tile([128, 512], F32R, name="zeros_cb" if _CACHE_BUST else "zeros")
            nc.vector.memset(zeros_t[:].bitcast(F32), 0.0)

            banks = [
                psp.tile([128, 512], F32, name=f"bank{k}", tag=f"bank{k}")
                for k in range(n_banks)
            ]
            def zero_bank(kb):
                nc.tensor.matmul(
                    banks[kb][:, :], zeros_t[:, 0:128], zeros_t[:, 0:512],
                    start=True, stop=False, skip_group_check=True,
                )

            for kb in range(n_banks):
                zero_bank(kb)

            # x rows: row(chunk k, partition p, g) = k*CHUNK + p*G + g
            x_view = x_d.rearrange("(k p g) c -> k p (g c)", p=128, g=G)

            for k in range(chunks):
                pow_t = powp.tile([128, 4 * GC], F32R)
                nc.sync.dma_start(out=pow_t[:, 0:GC], in_=x_view[k])
                if k % 4 == 0:
                    nb = min(4, chunks - k)
                    r4_t = relp.tile([128, 4 * G], F32, name="r4", tag="r4")
                    nc.sync.dma_start(
                        out=r4_t[:, 0 : nb * G],
                        in_=rel_d[:, k * G : (k + nb) * G],
                    )
                r_t = r4_t[:, (k % 4) * G : (k % 4 + 1) * G]

                x_g = pow_t[:, 0:GC].rearrange("p (g c) -> p g c", c=C)
                x2_g = pow_t[:, GC : 2 * GC].rearrange("p (g c) -> p g c", c=C)
                x3_g = pow_t[:, 2 * GC : 3 * GC].rearrange("p (g c) -> p g c", c=C)
                x4_g = pow_t[:, 3 * GC : 4 * GC].rearrange("p (g c) -> p g c", c=C)

                oh_t = ohp.tile([128, G, W], F32R)
                nc.vector.tensor_tensor(
                    out=oh_t[:, :, :],
                    in0=iota_t[:].unsqueeze(1).broadcast_to((128, G, W)),
                    in1=r_t.unsqueeze(2).broadcast_to((128, G, W)),
                    op=mybir.AluOpType.is_equal,
                )
                # Fused squares: one ACT pass over [x | x2] -> [x2 | x4].
                # Safe RAW: x2[i] is written GC elements before x4 reads it.
                sq_in = pow_t[:, 0 : 2 * GC].rearrange("p (s gc) -> p s gc", s=2)
                sq_out = bass.AP(
                    tensor=pow_t.tensor,
                    offset=pow_t[:].offset + GC,
                    ap=[pow_t[:].ap[0], [2 * GC, 2], [1, GC]],
                )
                nc.scalar.activation(
                    out=sq_out, in_=sq_in, func=mybir.ActivationFunctionType.Square
                )
                nc.vector.tensor_tensor(
                    out=x3_g, in0=x_g, in1=x2_g, op=mybir.AluOpType.mult
                )

                pow_4 = pow_t[:].rearrange("p (s gc) -> p s gc", s=4)
                for g in range(G):
                    t = k * G + g
                    w = t // t_win
                    bank = banks[w // 2]
                    col0 = (w % 2) * 256
                    bank_done = (t + 1) % (2 * t_win) == 0
                    nc.tensor.matmul(
                        bank[0:W, col0 : col0 + 256],
                        oh_t[:, g, :],
                        pow_4[:, :, g * C : (g + 1) * C],
                        start=False,
                        stop=bank_done,
                        skip_group_check=True,
                    )
                    if bank_done:
                        kb = w // 2
                        o_t = outp.tile([W, 512], F32, name=f"o{kb}", tag="o")
                        nc.vector.tensor_copy(o_t[:, :], bank[0:W, :])
                        row0 = kb * 2 * W
                        # single DMA: out rows row0+W*j+p <- o_t[p, j*256+c]
                        od = out_d[:]
                        out_ap = bass.AP(
                            tensor=od.tensor,
                            offset=od.offset + row0 * 256,
                            ap=[[256, W], [W * 256, 2], [1, 256]],
                        )
                        in_ap = o_t[:].rearrange("p (j c) -> p j c", j=2)
                        nc.sync.dma_start(out=out_ap, in_=in_ap)

    _split_excess_waits(nc, mybir)
    return nc


def _prepare_inputs(graph, batch_indices):
    idx = np.asarray(batch_indices).astype(np.int64)
    x = np.ascontiguousarray(np.asarray(graph, dtype=np.float32))
    n = idx.shape[0]

    counts = np.bincount(idx, minlength=B).astype(np.float64)

    n_windows = B // W
    bnd = np.searchsorted(idx, np.arange(0, B + 1, W))
    win_counts = np.diff(bnd)
    t_win = max(1, int(np.ceil(win_counts.max() / 128)))

    tiles = WINDOWS_PER_CORE * t_win
    rows = tiles * 128
    slots_per_win = t_win * 128

    src = np.full(n_windows * slots_per_win, -1, dtype=np.int64)
    base = np.repeat(np.arange(n_windows) * slots_per_win - bnd[:-1], win_counts)
    src[np.arange(n) + base] = np.arange(n)

    rel_flat = np.full(src.shape[0], -1.0, dtype=np.float32)
    valid = src >= 0
    winof = np.arange(src.shape[0]) // slots_per_win
    rel_flat[valid] = (idx[src[valid]] - W * winof[valid]).astype(np.float32)

    x_rounded = _round_fp32r(x)

    t_all = np.arange(rows) // 128
    p_all = np.arange(rows) % 128
    dram_row = (t_all // G) * CHUNK + p_all * G + (t_all % G)

    xs, rels = [], []
    for core in range(N_CORES):
        lo = core * WINDOWS_PER_CORE * slots_per_win
        hi = lo + WINDOWS_PER_CORE * slots_per_win
        csrc = src[lo:hi]
        crel = rel_flat[lo:hi]
        xl = np.zeros((rows, C), dtype=np.float32)
        cvalid = csrc >= 0
        xl[dram_row[cvalid]] = x_rounded[csrc[cvalid]]
        xs.append(xl)
        relT = np.empty((128, tiles), dtype=np.float32)
        relT[p_all, t_all] = crel
        rels.append(relT)
    iota = np.tile(np.arange(W, dtype=np.float32), (128, 1))
    return t_win, xs, rels, iota, counts


def _finalize(sums, counts):
    """sums: [B, 4C] raw power sums (S1|S2|S3|S4) -> [B, 4C] moments f32."""
    s = sums.astype(np.float64)
    ncnt = np.maximum(counts, 1.0)[:, None]
    M1 = s[:, 0:C] / ncnt
    M2 = s[:, C : 2 * C] / ncnt
    M3 = s[:, 2 * C : 3 * C] / ncnt
    M4 = s[:, 3 * C : 4 * C] / ncnt
    mean = M1
    var = M2 - M1 * M1
    skew = M3 - 3.0 * M1 * M2 + 2.0 * M1 * M1 * M1
    kurt = (
        M4
        - 4.0 * M1 * M3
        + 6.0 * M1 * M1 * M2
        - 3.0 * M1 * M1 * M1 * M1
        - 3.0
    )
    return np.concatenate([mean, var, skew, kurt], axis=1).astype(np.float32)


def kernel(graph, batch_indices):
    from concourse.bass_utils import run_bass_kernel_spmd

    t_win, xs, rels, iota, counts = _prepare_inputs(graph, batch_indices)
    if t_win not in _prog_cache:
        _prog_cache[t_win] = _build_program(t_win)
    nc = _prog_cache[t_win]
    in_maps = [
        {"x": xs[c], "rel": rels[c], "iota": iota} for c in range(N_CORES)
    ]
    res = run_bass_kernel_spmd(
        nc, in_maps, core_ids=list(range(N_CORES)), trace=TRACE
    )
    if TRACE:
        print(f"HW exec time: {res.exec_time_ns} ns")
        print(f"mean exec time: {res.mean_exec_time_ns} ns on slowest core "
              f"{res.max_exec_time_core_id}; trace: "
              f"{res.instructions_and_trace[1] if res.instructions_and_trace else None}")
    sums = np.concatenate([res.results[c]["out"] for c in range(N_CORES)], axis=0)
    return _finalize(sums, counts)



# revision 2
# speedup vs baseline: 1.0692x; 1.0692x over previous
"""Trainium2 Bass kernel: segmented statistical moments (mean/var/skew/kurt).

Strategy (8 NeuronCores, one SPMD program):
  - 4096 sorted segments -> 512 consecutive segments per core; host re-packs
    each core's nodes into a fixed program-uniform layout (so the single
    program works on all cores; per-node data differs, addressing does not).
  - Segments grouped into windows of W=32; every window padded to a common
    tile count t_win (multiple of G). Host precomputes the [128, tiles, 32]
    one-hot (bf16) directly — no on-device is_equal needed.
  - All node data flows in bf16: x is DMA'd as bf16; ACT computes x^2
    (Square), DVE computes x^3 = x*x2 and x^4 = x2*x2 (2x-packed bf16 mode).
  - Per 128-node tile: matmul onehot.T @ [x | x^2 | x^3 | x^4] (bf16,
    N=256, 1 cycle/row) accumulates per-segment power sums in f32 PSUM.
    Each window's first matmul uses start=True (no bank-zeroing needed);
    two windows share each PSUM bank (col 0/256).
  - Power sums copied out raw; host finalizes moments (float64) and
    concatenates cores.
"""

import sys

if "/opt/trn_rl_repo" not in sys.path:
    sys.path.insert(0, "/opt/trn_rl_repo")

import numpy as np
import ml_dtypes

BF16 = ml_dtypes.bfloat16

N_CORES = 8
B = 4096
C = 64
SEGS_PER_CORE = B // N_CORES      # 512
W = 32                            # segments per window
WINDOWS_PER_CORE = SEGS_PER_CORE // W   # 16
G = 16                            # 128-node tiles per chunk
GC = G * C                        # 1024
CHUNK = 128 * G                   # 2048 node slots per chunk

_prog_cache = {}
TRACE = False


def _split_excess_waits(nc, mybir, max_waits=1):
    """This walrus build allows only one sync-wait per instruction; move
    extra waits onto standalone sequencer EventSemaphore instructions."""
    n = [0]

    def mk(engine, waits):
        wi = mybir.InstEventSemaphore(name=f"xw_{n[0]}", ins=[], outs=[])
        n[0] += 1
        wi.engine = engine
        wi.sync_info = mybir.SyncInfo(on_wait=list(waits), on_update=[])
        return wi

    for bb in nc.main_func.blocks:
        out = []
        for ins in bb.instructions:
            if ins.opcode == "Matmult":
                # one-hot weights are exactly {0,1}
                ins.is_weight_onezero = True
            si = ins.sync_info
            if (
                si is not None
                and len(si.on_wait) > max_waits
                and ins.opcode != "EventSemaphore"
            ):
                waits = list(si.on_wait)
                for w in waits[:-max_waits]:
                    out.append(mk(ins.engine, [w]))
                ins.sync_info = mybir.SyncInfo(
                    on_wait=waits[-max_waits:], on_update=list(si.on_update)
                )
            out.append(ins)
        bb.instructions = out


def _build_program(t_win):
    import concourse.bass as bass
    import concourse.tile as tile
    import concourse.mybir as mybir

    F32 = mybir.dt.float32
    BF = mybir.dt.bfloat16

    tiles = WINDOWS_PER_CORE * t_win
    assert tiles % G == 0
    chunks = tiles // G
    rows = tiles * 128
    n_banks = WINDOWS_PER_CORE // 2   # two windows per PSUM bank

    nc = bass.Bass()
    x_d = nc.dram_tensor("x", [rows, C], BF, kind="ExternalInput")
    oh_d = nc.dram_tensor("oh", [128, tiles, W], BF, kind="ExternalInput")
    out_d = nc.dram_tensor("out", [SEGS_PER_CORE, 4 * C], F32, kind="ExternalOutput")

    with tile.TileContext(nc) as tc:
        with (
            tc.tile_pool(name="powp", bufs=9) as powp,
            tc.tile_pool(name="ohp", bufs=8) as ohp,
            tc.tile_pool(name="psp", bufs=1, space="PSUM") as psp,
            tc.tile_pool(name="outp", bufs=3) as outp,
        ):
            banks = [
                psp.tile([128, 512], F32, name=f"bank{k}", tag=f"bank{k}")
                for k in range(n_banks)
            ]

            # x rows: row(chunk k, partition p, g) = k*CHUNK + p*G + g
            x_view = x_d.rearrange("(k p g) c -> k p (g c)", p=128, g=G)

            for k in range(chunks):
                pow_t = powp.tile([128, 4 * GC], BF)
                nc.sync.dma_start(out=pow_t[:, 0:GC], in_=x_view[k])
                oh_t = ohp.tile([128, G, W], BF)
                nc.sync.dma_start(out=oh_t[:], in_=oh_d[:, k * G : (k + 1) * G, :])

                x_g = pow_t[:, 0:GC]
                x2_g = pow_t[:, GC : 2 * GC]
                x3_g = pow_t[:, 2 * GC : 3 * GC]
                x4_g = pow_t[:, 3 * GC : 4 * GC]

                nc.scalar.activation(
                    out=x2_g, in_=x_g, func=mybir.ActivationFunctionType.Square
                )
                nc.vector.tensor_tensor(
                    out=x3_g, in0=x_g, in1=x2_g, op=mybir.AluOpType.mult
                )
                nc.vector.tensor_tensor(
                    out=x4_g, in0=x2_g, in1=x2_g, op=mybir.AluOpType.mult
                )

                pow_4 = pow_t[:].rearrange("p (s gc) -> p s gc", s=4)
                for g in range(G):
                    t = k * G + g
                    w = t // t_win
                    bank = banks[w // 2]
                    col0 = (w % 2) * 256
                    win_start = t % t_win == 0
                    bank_done = (t + 1) % (2 * t_win) == 0
                    nc.tensor.matmul(
                        bank[0:W, col0 : col0 + 256],
                        oh_t[:, g, :],
                        pow_4[:, :, g * C : (g + 1) * C],
                        start=win_start,
                        stop=bank_done,
                        skip_group_check=True,
                    )
                    if bank_done:
                        kb = w // 2
                        o_t = outp.tile([W, 512], F32, name=f"o{kb}", tag="o")
                        nc.vector.tensor_copy(o_t[:, :], bank[0:W, :])
                        row0 = kb * 2 * W
                        # single DMA: out rows row0+W*j+p <- o_t[p, j*256+c]
                        od = out_d[:]
                        out_ap = bass.AP(
                            tensor=od.tensor,
                            offset=od.offset + row0 * 256,
                            ap=[[256, W], [W * 256, 2], [1, 256]],
                        )
                        in_ap = o_t[:].rearrange("p (j c) -> p j c", j=2)
                        nc.sync.dma_start(out=out_ap, in_=in_ap)

    _split_excess_waits(nc, mybir)
    return nc


def _prepare_inputs(graph, batch_indices):
    idx = np.asarray(batch_indices).astype(np.int64)
    x = np.ascontiguousarray(np.asarray(graph, dtype=np.float32))
    n = idx.shape[0]

    counts = np.bincount(idx, minlength=B).astype(np.float64)

    n_windows = B // W
    bnd = np.searchsorted(idx, np.arange(0, B + 1, W))
    win_counts = np.diff(bnd)
    t_win = max(1, int(np.ceil(win_counts.max() / 128)))

    tiles = WINDOWS_PER_CORE * t_win
    rows = tiles * 128
    slots_per_win = t_win * 128

    src = np.full(n_windows * slots_per_win, -1, dtype=np.int64)
    base = np.repeat(np.arange(n_windows) * slots_per_win - bnd[:-1], win_counts)
    src[np.arange(n) + base] = np.arange(n)

    rel_flat = np.full(src.shape[0], -1.0, dtype=np.float32)
    valid = src >= 0
    winof = np.arange(src.shape[0]) // slots_per_win
    rel_flat[valid] = (idx[src[valid]] - W * winof[valid]).astype(np.float32)

    x_bf = x.astype(BF16)

    t_all = np.arange(rows) // 128
    p_all = np.arange(rows) % 128
    dram_row = (t_all // G) * CHUNK + p_all * G + (t_all % G)

    xs, ohs = [], []
    wcol = np.arange(W, dtype=np.float32)
    for core in range(N_CORES):
        lo = core * WINDOWS_PER_CORE * slots_per_win
        hi = lo + WINDOWS_PER_CORE * slots_per_win
        csrc = src[lo:hi]
        crel = rel_flat[lo:hi]
        xl = np.zeros((rows, C), dtype=BF16)
        cvalid = csrc >= 0
        xl[dram_row[cvalid]] = x_bf[csrc[cvalid]]
        xs.append(xl)
        relT = np.empty((128, tiles), dtype=np.float32)
        relT[p_all, t_all] = crel
        oh = (relT[:, :, None] == wcol[None, None, :]).astype(BF16)
        ohs.append(oh)
    return t_win, xs, ohs, counts


def _finalize(sums, counts):
    """sums: [B, 4C] raw power sums (S1|S2|S3|S4) -> [B, 4C] moments f32."""
    s = sums.astype(np.float64)
    ncnt = np.maximum(counts, 1.0)[:, None]
    M1 = s[:, 0:C] / ncnt
    M2 = s[:, C : 2 * C] / ncnt
    M3 = s[:, 2 * C : 3 * C] / ncnt
    M4 = s[:, 3 * C : 4 * C] / ncnt
    mean = M1
    var = M2 - M1 * M1
    skew = M3 - 3.0 * M1 * M2 + 2.0 * M1 * M1 * M1
    kurt = (
        M4
        - 4.0 * M1 * M3
        + 6.0 * M1 * M1 * M2
        - 3.0 * M1 * M1 * M1 * M1
        - 3.0
    )
    return np.concatenate([mean, var, skew, kurt], axis=1).astype(np.float32)


def kernel(graph, batch_indices):
    from concourse.bass_utils import run_bass_kernel_spmd

    t_win, xs, ohs, counts = _prepare_inputs(graph, batch_indices)
    if t_win not in _prog_cache:
        _prog_cache[t_win] = _build_program(t_win)
    nc = _prog_cache[t_win]
    in_maps = [{"x": xs[c], "oh": ohs[c]} for c in range(N_CORES)]
    res = run_bass_kernel_spmd(
        nc, in_maps, core_ids=list(range(N_CORES)), trace=TRACE
    )
    if TRACE:
        print(f"HW exec time: {res.exec_time_ns} ns")
    sums = np.concatenate([res.results[c]["out"] for c in range(N_CORES)], axis=0)
    return _finalize(sums, counts)
